# revision 22
# baseline (speedup 1.0000x reference)
"""GroupedQueryAttention Trainium2 kernel (v2: transposed-S design).

Sharding: 8 cores = 2 (batch) x 4 (kv-head groups / tensor parallel).
Core c: b = c//4, g = c%4 owns q-heads 4g..4g+3 and kv-head g.
Each core computes a partial o-projection (its 512 rows of Wo); the host
sums the 4 partials per batch (the "all-reduce" of the TP group).

Device kernel per core, interleaved per 512-wide t-chunk j:
  1. proj(j): qT/kT/vT = W^T @ x^T in [head_dim, t] layout from a host
     pretransposed x^T.  v is PE-transposed to natural [s, d] layout.
  2. RoPE per chunk in [d, t] layout: the rotate-half (with sign) is a
     constant 128x128 permutation matmul on PE; cos/sin chunk tables are
     streamed from DRAM; multiplies/adds on DVE+Pool.
  3. attention per head in TRANSPOSED layout: S^T[s, t] = kT^T qT block
     matmuls; causal mask added on (extended) diagonal blocks; exp on ACT
     writes P^T straight PSUM->SBUF (softmax max-subtraction dropped:
     |S| <~ 7 here so fp32 exp is safe and the softmax ratio is exact).
     AV accumulates O^T[d, t] directly from P^T (no P transposes at all);
     the denominator accumulates via ones-vector matmuls in PSUM.
  4. 1/denom broadcast across partitions with a 1-partition ones matmul
     (no DRAM round-trip); O^T normalized to bf16.
  5. o-proj(j): y chunk = O^T^T @ Wo_shard (bf16), accumulated over the 4
     heads; partial y stored as bf16, summed on host in fp32.
"""

import math
import sys

import numpy as np
import ml_dtypes

sys.path.insert(0, "/opt/trn_rl_repo")

import concourse.bass as bass  # noqa: E402
import concourse.tile as tile  # noqa: E402
from concourse import bacc, mybir  # noqa: E402
from concourse.bass_utils import run_bass_kernel_spmd  # noqa: E402

B, T, D = 2, 2048, 2048
NH, NKV, HD = 16, 4, 128
NQ = NH // NKV  # q heads per core
KC = D // 128  # contraction chunks
NJ = T // 512  # t chunks
F32 = mybir.dt.float32
F32R = mybir.dt.float32r
BF16 = mybir.dt.bfloat16
X = mybir.AxisListType.X
EXP = mybir.ActivationFunctionType.Exp
COPY = mybir.ActivationFunctionType.Copy
NEGINF = -1.0e30


def _r(ap):
    return ap.bitcast(F32R)


def _c0_of(st, j):
    stl = st - 4 * j
    if stl < 0:
        return 0
    return (0, 128, 256, 256)[stl]


def _body(tc, xt, wq, wk, wv, wo, cost_d, sint_d, maskx_d, identd, rotmd, onesd, y_d):
    nc = tc.nc
    from contextlib import ExitStack

    with ExitStack() as ctx:
        consts = ctx.enter_context(tc.tile_pool(name="consts", bufs=1))
        wpool = ctx.enter_context(tc.tile_pool(name="wpool", bufs=1))
        kv = ctx.enter_context(tc.tile_pool(name="kv", bufs=1))
        xp = ctx.enter_context(tc.tile_pool(name="xp", bufs=8))
        qp = ctx.enter_context(tc.tile_pool(name="qp", bufs=8))
        rt = ctx.enter_context(tc.tile_pool(name="rt", bufs=3))
        cs = ctx.enter_context(tc.tile_pool(name="cs", bufs=2))
        ptp = ctx.enter_context(tc.tile_pool(name="ptp", bufs=5))
        otp = ctx.enter_context(tc.tile_pool(name="otp", bufs=6))
        ivp = ctx.enter_context(tc.tile_pool(name="ivp", bufs=1))
        ibp_pool = ctx.enter_context(tc.tile_pool(name="ibp", bufs=2))
        ysp = ctx.enter_context(tc.tile_pool(name="ysp", bufs=2))
        ps = ctx.enter_context(tc.tile_pool(name="ps", bufs=1, space="PSUM"))

        def load_x(j):
            tiles = []
            for q4 in range(4):
                xtile = xp.tile([128, 4, 512], F32R, tag="x", name=f"x{j}_{q4}")
                nc.sync.dma_start(
                    xtile,
                    xt[512 * q4 : 512 * (q4 + 1), 512 * j : 512 * (j + 1)].rearrange(
                        "(c p) m -> p c m", p=128
                    ),
                )
                tiles.append(xtile)
            return tiles

        def load_cs(j):
            cosc = cs.tile([128, 512], F32R, tag="cos", name=f"cos{j}")
            nc.sync.dma_start(cosc, cost_d[:, 512 * j : 512 * (j + 1)])
            sinc = cs.tile([128, 512], F32R, tag="sin", name=f"sin{j}")
            nc.sync.dma_start(sinc, sint_d[:, 512 * j : 512 * (j + 1)])
            return cosc, sinc

        # ---- startup loads, ordered so proj(0) can start ASAP:
        # proj m=4 (k) consumes x quarters in order, needing only wk first.
        wkt = wpool.tile([128, 16, 128], F32R, tag="wk", bufs=1, name="wkt")
        nc.sync.dma_start(
            wkt[:, 0:4, :], wk[0:512, :].rearrange("(c p) m -> p c m", p=128)
        )
        xcur = [xp.tile([128, 4, 512], F32R, tag="x", name=f"x0_{q4}") for q4 in range(4)]
        nc.sync.dma_start(
            xcur[0], xt[0:512, 0:512].rearrange("(c p) m -> p c m", p=128)
        )
        nc.sync.dma_start(
            wkt[:, 4:16, :], wk[512:2048, :].rearrange("(c p) m -> p c m", p=128)
        )
        for q4 in (1, 2, 3):
            nc.sync.dma_start(
                xcur[q4],
                xt[512 * q4 : 512 * (q4 + 1), 0:512].rearrange("(c p) m -> p c m", p=128),
            )
        wvt = wpool.tile([128, 16, 128], F32R, tag="wv", bufs=1, name="wvt")
        nc.sync.dma_start(wvt, wv.rearrange("(c p) m -> p c m", p=128))
        ident = consts.tile([128, 128], F32R, name="ident")
        nc.sync.dma_start(ident, identd)
        wqt = []
        for i in range(4):
            w = wpool.tile([128, 16, 128], F32R, tag="wq", bufs=4, name=f"wq{i}")
            nc.sync.dma_start(
                w,
                wq[:, 128 * i : 128 * (i + 1)].rearrange("(c p) m -> p c m", p=128),
            )
            wqt.append(w)
        rotm = consts.tile([128, 128], F32R, name="rotm")
        nc.sync.dma_start(rotm, rotmd)
        cscur = load_cs(0)
        maskx = consts.tile([128, 256], F32, name="maskx")
        nc.sync.dma_start(maskx, maskx_d)
        ones = consts.tile([128, 128], F32R, name="ones")
        nc.sync.dma_start(ones, onesd)
        wot = []
        for hh in range(4):
            w = wpool.tile([128, T], BF16, tag="wo", bufs=4, name=f"wo{hh}")
            nc.sync.dma_start(w, wo[128 * hh : 128 * (hh + 1), :])
            wot.append(w)

        kT = kv.tile([128, T], F32R, tag="kT", name="kT")
        vnat = kv.tile([128, T], F32R, tag="vnat", name="vnat")

        for j in range(NJ):
            jlo = 512 * j
            cosc, sinc = cscur
            qcur = [None] * 4

            # ---- proj(j) with RoPE inlined: each cross-engine dependency
            # gets a full 16-matmul block of PE slack before its consumer.
            def proj_block(m):
                pm = ps.tile([128, 512], F32, tag="big", bufs=5, name=f"pm{j}_{m}")
                for kc in range(KC):
                    if m == 4:
                        lhsT = wkt[:, kc, :]
                    elif m == 5:
                        lhsT = wvt[:, kc, :]
                    else:
                        lhsT = wqt[m][:, kc, :]
                    nc.tensor.matmul(
                        pm,
                        lhsT,
                        xcur[kc // 4][:, kc % 4, :],
                        start=(kc == 0),
                        stop=(kc == KC - 1),
                    )
                if m == 4:
                    nc.vector.tensor_copy(kT[:, jlo : jlo + 512], pm)
                elif m == 5:
                    vtmp_ = rt.tile([128, 512], F32R, tag="rt", name=f"vtmp{j}")
                    nc.vector.tensor_copy(vtmp_, pm)
                    return vtmp_
                else:
                    qc = qp.tile([128, 512], F32R, tag="qt", name=f"q{j}_{m}")
                    nc.scalar.activation(qc, pm, COPY)
                    qcur[m] = qc
                return None

            def rope(tgt, ri):
                rp = ps.tile([128, 512], F32, tag="big", bufs=5, name=f"rot{j}_{ri}")
                nc.tensor.matmul(rp, rotm, tgt)
                tmp = rt.tile([128, 512], F32R, tag="rt", name=f"rtmp{j}_{ri}")
                nc.vector.tensor_mul(tmp, rp, sinc)
                nc.gpsimd.tensor_mul(tgt, tgt, cosc)
                nc.vector.tensor_add(tgt, tgt, tmp)

            proj_block(4)
            vtmp = proj_block(5)
            proj_block(0)
            rope(kT[:, jlo : jlo + 512], "k")
            proj_block(1)
            rope(qcur[0], "q0")
            for c in range(4):
                tp = ps.tile([128, 128], F32, tag="big", bufs=5, name=f"vt{j}{c}")
                nc.tensor.transpose(_r(tp), vtmp[:, 128 * c : 128 * (c + 1)], ident)
                st = 4 * j + c
                nc.vector.tensor_copy(vnat[:, 128 * st : 128 * (st + 1)], tp)
            proj_block(2)
            rope(qcur[1], "q1")
            proj_block(3)
            rope(qcur[2], "q2")
            rope(qcur[3], "q3")

            # prefetch next chunk's x and rope tables
            if j + 1 < NJ:
                xnext = load_x(j + 1)
                csnext = load_cs(j + 1)

            # ---- attention per head, transposed layout ----
            nb = 4 * j + 4
            otcur = [None] * 4

            def emit_epilogue(pend):
                # deferred normalization: 1/den broadcast + O^T scale to bf16
                hh_, inv_p, avp_ = pend
                ibt = ps.tile([128, 512], F32, tag="big", bufs=5, name=f"ib{j}_{hh_}")
                nc.tensor.matmul(ibt, ones[0:1, :].bitcast(F32), inv_p[0:1, :])
                ivb = ibp_pool.tile([128, 512], F32, tag="invb", name=f"ivb{j}_{hh_}")
                nc.vector.tensor_copy(ivb, ibt)
                otc = otp.tile([128, 512], BF16, tag="ot", name=f"ot{j}_{hh_}")
                nc.vector.tensor_mul(otc, avp_, ivb)
                otcur[hh_] = otc

            pending = None
            for h in range(NQ):
                avp = ps.tile([128, 512], F32, tag="av", bufs=2, name=f"av{j}_{h}")
                dnp = ps.tile([128, 512], F32, tag="dn", bufs=1, name=f"dn{j}_{h}")
                pts = {}

                def emit_s(st):
                    c0 = _c0_of(st, j)
                    sp = ps.tile([128, 512], F32, tag="big", bufs=5, name=f"s{j}{h}{st}")
                    nc.tensor.matmul(
                        sp[:, c0:512],
                        kT[:, 128 * st : 128 * (st + 1)],
                        qcur[h][:, c0:512],
                    )
                    stl = st - 4 * j
                    if stl == 3:
                        nc.vector.tensor_add(sp[:, 256:512], sp[:, 256:512], maskx)
                    elif stl >= 0:
                        od = 128 * stl
                        nc.vector.tensor_add(
                            sp[:, od : od + 128], sp[:, od : od + 128], maskx[:, 128:256]
                        )
                    pt_ = ptp.tile([128, 512], F32R, tag="pt", name=f"p{j}{h}{st}")
                    nc.scalar.activation(pt_[:, c0:512], sp[:, c0:512], EXP)
                    pts[st] = pt_

                emit_s(0)
                emit_s(1)
                emit_s(2)
                emit_s(3)
                for st in range(nb):
                    if st + 4 < nb:
                        emit_s(st + 4)
                    if st == 1 and pending is not None:
                        emit_epilogue(pending)
                        pending = None
                    c0 = _c0_of(st, j)
                    nc.tensor.matmul(
                        dnp[0:1, c0:512],
                        ones[:, 0:1],
                        pts[st][:, c0:512],
                        start=(st == 0),
                        stop=(st == nb - 1),
                    )
                    nc.tensor.matmul(
                        avp[:, c0:512],
                        vnat[:, 128 * st : 128 * (st + 1)],
                        pts[st][:, c0:512],
                        start=(st == 0),
                        stop=(st == nb - 1),
                    )
                    del pts[st]

                inv_ = ivp.tile([1, 512], F32, tag="inv", name=f"inv{j}_{h}")
                nc.vector.reciprocal(inv_[0:1, :], dnp[0:1, :])
                pending = (h, inv_, avp)

            emit_epilogue(pending)
            pending = None

            # ---- o-proj(j): y rows [jlo, jlo+512) ----
            # il=0 runs hh-major so the 4 concurrent psum accumulations absorb
            # the last head's deferred-normalization latency; later ils run
            # nch-major so each psum frees (and its copy starts) early.
            for il in range(4):
                ysb = ysp.tile([128, T], BF16, tag="ysb", name=f"y{j}_{il}")

                def ycopy(nch, yp):
                    eng = nc.vector if nch % 2 == 0 else nc.scalar
                    if eng is nc.vector:
                        nc.vector.tensor_copy(
                            ysb[:, 512 * nch : 512 * (nch + 1)], yp
                        )
                    else:
                        nc.scalar.activation(
                            ysb[:, 512 * nch : 512 * (nch + 1)], yp, COPY
                        )

                if il == 0:
                    yps = [
                        ps.tile([128, 512], F32, tag="big", bufs=5, name=f"yp{j}0{n}")
                        for n in range(4)
                    ]
                    for hh in range(4):
                        for nch in range(4):
                            nc.tensor.matmul(
                                yps[nch],
                                otcur[hh][:, 0:128],
                                wot[hh][:, 512 * nch : 512 * (nch + 1)],
                                start=(hh == 0),
                                stop=(hh == 3),
                            )
                    for nch in range(4):
                        ycopy(nch, yps[nch])
                else:
                    for nch in range(4):
                        yp = ps.tile(
                            [128, 512], F32, tag="big", bufs=5, name=f"yp{j}{il}{nch}"
                        )
                        for hh in range(4):
                            nc.tensor.matmul(
                                yp,
                                otcur[hh][:, 128 * il : 128 * (il + 1)],
                                wot[hh][:, 512 * nch : 512 * (nch + 1)],
                                start=(hh == 0),
                                stop=(hh == 3),
                            )
                        ycopy(nch, yp)
                if j == NJ - 1 and il == 3:
                    nc.sync.dma_start(
                        y_d[jlo + 128 * il : jlo + 128 * (il + 1), 0:1024],
                        ysb[:, 0:1024],
                    )
                    nc.sync.dma_start(
                        y_d[jlo + 128 * il : jlo + 128 * (il + 1), 1024:2048],
                        ysb[:, 1024:2048],
                    )
                else:
                    nc.sync.dma_start(
                        y_d[jlo + 128 * il : jlo + 128 * (il + 1), :], ysb
                    )

            if j + 1 < NJ:
                xcur = xnext
                cscur = csnext


def build_nc():
    nc = bacc.Bacc("TRN2", target_bir_lowering=False, debug=False, num_devices=8)
    xt = nc.dram_tensor("xt", [D, T], F32R, kind="ExternalInput").ap()
    wq = nc.dram_tensor("wq", [D, NQ * HD], F32R, kind="ExternalInput").ap()
    wk = nc.dram_tensor("wk", [D, HD], F32R, kind="ExternalInput").ap()
    wv = nc.dram_tensor("wv", [D, HD], F32R, kind="ExternalInput").ap()
    wo = nc.dram_tensor("wo", [NQ * HD, D], BF16, kind="ExternalInput").ap()
    identd = nc.dram_tensor("identd", [128, 128], F32R, kind="ExternalInput").ap()
    rotmd = nc.dram_tensor("rotmd", [128, 128], F32R, kind="ExternalInput").ap()
    onesd = nc.dram_tensor("onesd", [128, 128], F32R, kind="ExternalInput").ap()
    cost = nc.dram_tensor("cost", [HD, T], F32R, kind="ExternalInput").ap()
    sint = nc.dram_tensor("sint", [HD, T], F32R, kind="ExternalInput").ap()
    maskx = nc.dram_tensor("maskx", [128, 256], F32, kind="ExternalInput").ap()
    y = nc.dram_tensor("y", [T, D], BF16, kind="ExternalOutput").ap()
    with tile.TileContext(nc) as tc:
        _body(tc, xt, wq, wk, wv, wo, cost, sint, maskx, identd, rotmd, onesd, y)
    nc.compile()
    return nc


def rope_tables():
    """cos/sin tables in [d, t] layout, NO sign folding (sign is in rotm)."""
    inv_freq = 1.0 / (10000.0 ** (np.arange(0, HD, 2, dtype=np.float32) / HD))
    t = np.arange(T, dtype=np.float32)
    freqs = t[:, None] * inv_freq[None, :]
    emb = np.concatenate([freqs, freqs], axis=1)  # [T, 128]
    cos = np.ascontiguousarray(np.cos(emb).T).astype(np.float32)
    sin = np.ascontiguousarray(np.sin(emb).T).astype(np.float32)
    return cos, sin


def rot_matrix():
    """rotm[k, m]: out[m] = sum_k rotm[k, m] q[k] = rotate_half(q)[m]."""
    r = np.zeros((128, 128), np.float32)
    for m in range(64):
        r[m + 64, m] = -1.0
    for m in range(64, 128):
        r[m - 64, m] = 1.0
    return r


def mask_ext():
    """[128, 256]: cols 0-127 fully masked; cols 128-255 causal triangle."""
    m = np.full((128, 256), NEGINF, np.float32)
    sl = np.arange(128)
    tl = np.arange(128)
    m[:, 128:] = np.where(sl[:, None] <= tl[None, :], 0.0, NEGINF)
    return m


def make_in_maps(x, Wq, Wk, Wv, Wo):
    scale = np.float32(1.0 / math.sqrt(HD))
    cos, sin = rope_tables()
    in_maps = []
    for c in range(8):
        b, g = c // 4, c % 4
        in_maps.append(
            {
                "xt": np.ascontiguousarray(x[b].T),
                "wq": np.ascontiguousarray(Wq[:, 512 * g : 512 * (g + 1)]) * scale,
                "wk": np.ascontiguousarray(Wk[:, 128 * g : 128 * (g + 1)]),
                "wv": np.ascontiguousarray(Wv[:, 128 * g : 128 * (g + 1)]),
                "wo": np.ascontiguousarray(Wo[512 * g : 512 * (g + 1), :]).astype(
                    ml_dtypes.bfloat16
                ),
                "cost": cos,
                "sint": sin,
                "maskx": mask_ext(),
                "identd": np.eye(128, dtype=np.float32),
                "onesd": np.ones((128, 128), np.float32),
                "rotmd": rot_matrix(),
            }
        )
    return in_maps


_CACHE = {}


def _get_nc():
    if "nc" not in _CACHE:
        _CACHE["nc"] = build_nc()
    return _CACHE["nc"]


def kernel(**inputs):
    x = np.asarray(inputs["x"], np.float32)
    Wq = np.asarray(inputs["Wq"], np.float32)
    Wk = np.asarray(inputs["Wk"], np.float32)
    Wv = np.asarray(inputs["Wv"], np.float32)
    Wo = np.asarray(inputs["Wo"], np.float32)
    in_maps = make_in_maps(x, Wq, Wk, Wv, Wo)
    nc = _get_nc()
    res = run_bass_kernel_spmd(nc, in_maps, core_ids=list(range(8)))
    outs = [np.asarray(r["y"]).astype(np.float32) for r in res.results]
    y = np.stack(
        [
            outs[0] + outs[1] + outs[2] + outs[3],
            outs[4] + outs[5] + outs[6] + outs[7],
        ]
    )
    return y.astype(np.float32)


# revision 23
# speedup vs baseline: 1.0365x; 1.0365x over previous
"""GroupedQueryAttention Trainium2 kernel (v2: transposed-S design).

Sharding: 8 cores = 2 (batch) x 4 (kv-head groups / tensor parallel).
Core c: b = c//4, g = c%4 owns q-heads 4g..4g+3 and kv-head g.
Each core computes a partial o-projection (its 512 rows of Wo); the host
sums the 4 partials per batch (the "all-reduce" of the TP group).

Device kernel per core, interleaved per 512-wide t-chunk j:
  1. proj(j): qT/kT/vT = W^T @ x^T in [head_dim, t] layout from a host
     pretransposed x^T.  v is PE-transposed to natural [s, d] layout.
  2. RoPE per chunk in [d, t] layout: the rotate-half (with sign) is a
     constant 128x128 permutation matmul on PE; cos/sin chunk tables are
     streamed from DRAM; multiplies/adds on DVE+Pool.
  3. attention per head in TRANSPOSED layout: S^T[s, t] = kT^T qT block
     matmuls; causal mask added on (extended) diagonal blocks; exp on ACT
     writes P^T straight PSUM->SBUF (softmax max-subtraction dropped:
     |S| <~ 7 here so fp32 exp is safe and the softmax ratio is exact).
     AV accumulates O^T[d, t] directly from P^T (no P transposes at all);
     the denominator accumulates via ones-vector matmuls in PSUM.
  4. 1/denom broadcast across partitions with a 1-partition ones matmul
     (no DRAM round-trip); O^T normalized to bf16.
  5. o-proj(j): y chunk = O^T^T @ Wo_shard (bf16), accumulated over the 4
     heads; partial y stored as bf16, summed on host in fp32.
"""

import math
import sys

import numpy as np
import ml_dtypes

sys.path.insert(0, "/opt/trn_rl_repo")

import concourse.bass as bass  # noqa: E402
import concourse.tile as tile  # noqa: E402
from concourse import bacc, mybir  # noqa: E402
from concourse.bass_utils import run_bass_kernel_spmd  # noqa: E402

B, T, D = 2, 2048, 2048
NH, NKV, HD = 16, 4, 128
NQ = NH // NKV  # q heads per core
KC = D // 128  # contraction chunks
NJ = T // 512  # t chunks
F32 = mybir.dt.float32
F32R = mybir.dt.float32r
BF16 = mybir.dt.bfloat16
X = mybir.AxisListType.X
EXP = mybir.ActivationFunctionType.Exp
COPY = mybir.ActivationFunctionType.Copy
NEGINF = -1.0e30


def _r(ap):
    return ap.bitcast(F32R)


def _c0_of(st, j):
    stl = st - 4 * j
    if stl < 0:
        return 0
    return (0, 128, 256, 256)[stl]


def _body(tc, xt, wq, wk, wv, wo, cost_d, sint_d, maskx_d, identd, rotmd, onesd, y_d):
    nc = tc.nc
    from contextlib import ExitStack

    with ExitStack() as ctx:
        consts = ctx.enter_context(tc.tile_pool(name="consts", bufs=1))
        wpool = ctx.enter_context(tc.tile_pool(name="wpool", bufs=1))
        kv = ctx.enter_context(tc.tile_pool(name="kv", bufs=1))
        xp = ctx.enter_context(tc.tile_pool(name="xp", bufs=8))
        qp = ctx.enter_context(tc.tile_pool(name="qp", bufs=8))
        rt = ctx.enter_context(tc.tile_pool(name="rt", bufs=3))
        cs = ctx.enter_context(tc.tile_pool(name="cs", bufs=2))
        ptp = ctx.enter_context(tc.tile_pool(name="ptp", bufs=5))
        otp = ctx.enter_context(tc.tile_pool(name="otp", bufs=6))
        ivp = ctx.enter_context(tc.tile_pool(name="ivp", bufs=1))
        ibp_pool = ctx.enter_context(tc.tile_pool(name="ibp", bufs=2))
        ysp = ctx.enter_context(tc.tile_pool(name="ysp", bufs=2))
        ps = ctx.enter_context(tc.tile_pool(name="ps", bufs=1, space="PSUM"))

        def load_x(j):
            tiles = []
            for q4 in range(4):
                xtile = xp.tile([128, 4, 512], F32R, tag="x", name=f"x{j}_{q4}")
                nc.sync.dma_start(
                    xtile,
                    xt[512 * q4 : 512 * (q4 + 1), 512 * j : 512 * (j + 1)].rearrange(
                        "(c p) m -> p c m", p=128
                    ),
                )
                tiles.append(xtile)
            return tiles

        def load_cs(j):
            cosc = cs.tile([128, 512], F32R, tag="cos", name=f"cos{j}")
            nc.sync.dma_start(cosc, cost_d[:, 512 * j : 512 * (j + 1)])
            sinc = cs.tile([128, 512], F32R, tag="sin", name=f"sin{j}")
            nc.sync.dma_start(sinc, sint_d[:, 512 * j : 512 * (j + 1)])
            return cosc, sinc

        # ---- startup loads, ordered so proj(0) can start ASAP:
        # proj m=4 (k) consumes x quarters in order, needing only wk first.
        wkt = wpool.tile([128, 16, 128], F32R, tag="wk", bufs=1, name="wkt")
        nc.sync.dma_start(
            wkt[:, 0:4, :], wk[0:512, :].rearrange("(c p) m -> p c m", p=128)
        )
        xcur = [xp.tile([128, 4, 512], F32R, tag="x", name=f"x0_{q4}") for q4 in range(4)]
        nc.sync.dma_start(
            xcur[0], xt[0:512, 0:512].rearrange("(c p) m -> p c m", p=128)
        )
        nc.sync.dma_start(
            wkt[:, 4:16, :], wk[512:2048, :].rearrange("(c p) m -> p c m", p=128)
        )
        for q4 in (1, 2, 3):
            nc.sync.dma_start(
                xcur[q4],
                xt[512 * q4 : 512 * (q4 + 1), 0:512].rearrange("(c p) m -> p c m", p=128),
            )
        wvt = wpool.tile([128, 16, 128], F32R, tag="wv", bufs=1, name="wvt")
        nc.sync.dma_start(wvt, wv.rearrange("(c p) m -> p c m", p=128))
        ident = consts.tile([128, 128], F32R, name="ident")
        nc.sync.dma_start(ident, identd)
        wqt = []
        for i in range(4):
            w = wpool.tile([128, 4, 512], F32R, tag="wq", bufs=4, name=f"wq{i}")
            nc.sync.dma_start(
                w, wq[512 * i : 512 * (i + 1), :].rearrange("(c p) m -> p c m", p=128)
            )
            wqt.append(w)
        rotm = consts.tile([128, 128], F32R, name="rotm")
        nc.sync.dma_start(rotm, rotmd)
        cscur = load_cs(0)
        maskx = consts.tile([128, 256], F32, name="maskx")
        nc.sync.dma_start(maskx, maskx_d)
        ones = consts.tile([128, 128], F32R, name="ones")
        nc.sync.dma_start(ones, onesd)
        wot = []
        for hh in range(4):
            w = wpool.tile([128, T], BF16, tag="wo", bufs=4, name=f"wo{hh}")
            nc.sync.dma_start(w, wo[128 * hh : 128 * (hh + 1), :])
            wot.append(w)

        kT = kv.tile([128, T], F32R, tag="kT", name="kT")
        vnat = kv.tile([128, T], F32R, tag="vnat", name="vnat")

        for j in range(NJ):
            jlo = 512 * j
            cosc, sinc = cscur
            qcur = [None] * 4

            # ---- proj(j) with RoPE inlined: each cross-engine dependency
            # gets a full 16-matmul block of PE slack before its consumer.
            def proj_block(m):
                pm = ps.tile([128, 512], F32, tag="big", bufs=5, name=f"pm{j}_{m}")
                for kc in range(KC):
                    if m == 4:
                        lhsT = wkt[:, kc, :]
                    elif m == 5:
                        lhsT = wvt[:, kc, :]
                    else:
                        lhsT = wqt[kc // 4][:, kc % 4, 128 * m : 128 * (m + 1)]
                    nc.tensor.matmul(
                        pm,
                        lhsT,
                        xcur[kc // 4][:, kc % 4, :],
                        start=(kc == 0),
                        stop=(kc == KC - 1),
                    )
                if m == 4:
                    nc.vector.tensor_copy(kT[:, jlo : jlo + 512], pm)
                elif m == 5:
                    vtmp_ = rt.tile([128, 512], F32R, tag="rt", name=f"vtmp{j}")
                    nc.vector.tensor_copy(vtmp_, pm)
                    return vtmp_
                else:
                    qc = qp.tile([128, 512], F32R, tag="qt", name=f"q{j}_{m}")
                    nc.scalar.activation(qc, pm, COPY)
                    qcur[m] = qc
                return None

            def rope(tgt, ri):
                rp = ps.tile([128, 512], F32, tag="big", bufs=5, name=f"rot{j}_{ri}")
                nc.tensor.matmul(rp, rotm, tgt)
                tmp = rt.tile([128, 512], F32R, tag="rt", name=f"rtmp{j}_{ri}")
                nc.vector.tensor_mul(tmp, rp, sinc)
                nc.gpsimd.tensor_mul(tgt, tgt, cosc)
                nc.vector.tensor_add(tgt, tgt, tmp)

            proj_block(4)
            vtmp = proj_block(5)
            proj_block(0)
            rope(kT[:, jlo : jlo + 512], "k")
            proj_block(1)
            rope(qcur[0], "q0")
            for c in range(4):
                tp = ps.tile([128, 128], F32, tag="big", bufs=5, name=f"vt{j}{c}")
                nc.tensor.transpose(_r(tp), vtmp[:, 128 * c : 128 * (c + 1)], ident)
                st = 4 * j + c
                nc.vector.tensor_copy(vnat[:, 128 * st : 128 * (st + 1)], tp)
            proj_block(2)
            rope(qcur[1], "q1")
            proj_block(3)
            rope(qcur[2], "q2")
            rope(qcur[3], "q3")

            # prefetch next chunk's x and rope tables
            if j + 1 < NJ:
                xnext = load_x(j + 1)
                csnext = load_cs(j + 1)

            # ---- attention per head, transposed layout ----
            nb = 4 * j + 4
            otcur = [None] * 4

            def emit_epilogue(pend):
                # deferred normalization: 1/den broadcast + O^T scale to bf16
                hh_, inv_p, avp_ = pend
                ibt = ps.tile([128, 512], F32, tag="big", bufs=5, name=f"ib{j}_{hh_}")
                nc.tensor.matmul(ibt, ones[0:1, :].bitcast(F32), inv_p[0:1, :])
                ivb = ibp_pool.tile([128, 512], F32, tag="invb", name=f"ivb{j}_{hh_}")
                nc.vector.tensor_copy(ivb, ibt)
                otc = otp.tile([128, 512], BF16, tag="ot", name=f"ot{j}_{hh_}")
                nc.vector.tensor_mul(otc, avp_, ivb)
                otcur[hh_] = otc

            pending = None
            for h in range(NQ):
                avp = ps.tile([128, 512], F32, tag="av", bufs=2, name=f"av{j}_{h}")
                dnp = ps.tile([128, 512], F32, tag="dn", bufs=1, name=f"dn{j}_{h}")
                pts = {}

                def emit_s(st):
                    c0 = _c0_of(st, j)
                    sp = ps.tile([128, 512], F32, tag="big", bufs=5, name=f"s{j}{h}{st}")
                    nc.tensor.matmul(
                        sp[:, c0:512],
                        kT[:, 128 * st : 128 * (st + 1)],
                        qcur[h][:, c0:512],
                    )
                    stl = st - 4 * j
                    if stl == 3:
                        nc.vector.tensor_add(sp[:, 256:512], sp[:, 256:512], maskx)
                    elif stl >= 0:
                        od = 128 * stl
                        nc.vector.tensor_add(
                            sp[:, od : od + 128], sp[:, od : od + 128], maskx[:, 128:256]
                        )
                    pt_ = ptp.tile([128, 512], F32R, tag="pt", name=f"p{j}{h}{st}")
                    nc.scalar.activation(pt_[:, c0:512], sp[:, c0:512], EXP)
                    pts[st] = pt_

                emit_s(0)
                emit_s(1)
                emit_s(2)
                emit_s(3)
                for st in range(nb):
                    if st + 4 < nb:
                        emit_s(st + 4)
                    if st == 1 and pending is not None:
                        emit_epilogue(pending)
                        pending = None
                    c0 = _c0_of(st, j)
                    nc.tensor.matmul(
                        dnp[0:1, c0:512],
                        ones[:, 0:1],
                        pts[st][:, c0:512],
                        start=(st == 0),
                        stop=(st == nb - 1),
                    )
                    nc.tensor.matmul(
                        avp[:, c0:512],
                        vnat[:, 128 * st : 128 * (st + 1)],
                        pts[st][:, c0:512],
                        start=(st == 0),
                        stop=(st == nb - 1),
                    )
                    del pts[st]

                inv_ = ivp.tile([1, 512], F32, tag="inv", name=f"inv{j}_{h}")
                nc.vector.reciprocal(inv_[0:1, :], dnp[0:1, :])
                pending = (h, inv_, avp)

            emit_epilogue(pending)
            pending = None

            # ---- o-proj(j): y rows [jlo, jlo+512) ----
            # il=0 runs hh-major so the 4 concurrent psum accumulations absorb
            # the last head's deferred-normalization latency; later ils run
            # nch-major so each psum frees (and its copy starts) early.
            for il in range(4):
                ysb = ysp.tile([128, T], BF16, tag="ysb", name=f"y{j}_{il}")

                def ycopy(nch, yp):
                    eng = nc.vector if nch % 2 == 0 else nc.scalar
                    if eng is nc.vector:
                        nc.vector.tensor_copy(
                            ysb[:, 512 * nch : 512 * (nch + 1)], yp
                        )
                    else:
                        nc.scalar.activation(
                            ysb[:, 512 * nch : 512 * (nch + 1)], yp, COPY
                        )

                if il == 0:
                    yps = [
                        ps.tile([128, 512], F32, tag="big", bufs=5, name=f"yp{j}0{n}")
                        for n in range(4)
                    ]
                    for hh in range(4):
                        for nch in range(4):
                            nc.tensor.matmul(
                                yps[nch],
                                otcur[hh][:, 0:128],
                                wot[hh][:, 512 * nch : 512 * (nch + 1)],
                                start=(hh == 0),
                                stop=(hh == 3),
                            )
                    for nch in range(4):
                        ycopy(nch, yps[nch])
                else:
                    for nch in range(4):
                        yp = ps.tile(
                            [128, 512], F32, tag="big", bufs=5, name=f"yp{j}{il}{nch}"
                        )
                        for hh in range(4):
                            nc.tensor.matmul(
                                yp,
                                otcur[hh][:, 128 * il : 128 * (il + 1)],
                                wot[hh][:, 512 * nch : 512 * (nch + 1)],
                                start=(hh == 0),
                                stop=(hh == 3),
                            )
                        ycopy(nch, yp)
                if j == NJ - 1 and il == 3:
                    nc.sync.dma_start(
                        y_d[jlo + 128 * il : jlo + 128 * (il + 1), 0:1024],
                        ysb[:, 0:1024],
                    )
                    nc.sync.dma_start(
                        y_d[jlo + 128 * il : jlo + 128 * (il + 1), 1024:2048],
                        ysb[:, 1024:2048],
                    )
                else:
                    nc.sync.dma_start(
                        y_d[jlo + 128 * il : jlo + 128 * (il + 1), :], ysb
                    )

            if j + 1 < NJ:
                xcur = xnext
                cscur = csnext


def build_nc():
    nc = bacc.Bacc("TRN2", target_bir_lowering=False, debug=False, num_devices=8)
    xt = nc.dram_tensor("xt", [D, T], F32R, kind="ExternalInput").ap()
    wq = nc.dram_tensor("wq", [D, NQ * HD], F32R, kind="ExternalInput").ap()
    wk = nc.dram_tensor("wk", [D, HD], F32R, kind="ExternalInput").ap()
    wv = nc.dram_tensor("wv", [D, HD], F32R, kind="ExternalInput").ap()
    wo = nc.dram_tensor("wo", [NQ * HD, D], BF16, kind="ExternalInput").ap()
    identd = nc.dram_tensor("identd", [128, 128], F32R, kind="ExternalInput").ap()
    rotmd = nc.dram_tensor("rotmd", [128, 128], F32R, kind="ExternalInput").ap()
    onesd = nc.dram_tensor("onesd", [128, 128], F32R, kind="ExternalInput").ap()
    cost = nc.dram_tensor("cost", [HD, T], F32R, kind="ExternalInput").ap()
    sint = nc.dram_tensor("sint", [HD, T], F32R, kind="ExternalInput").ap()
    maskx = nc.dram_tensor("maskx", [128, 256], F32, kind="ExternalInput").ap()
    y = nc.dram_tensor("y", [T, D], BF16, kind="ExternalOutput").ap()
    with tile.TileContext(nc) as tc:
        _body(tc, xt, wq, wk, wv, wo, cost, sint, maskx, identd, rotmd, onesd, y)
    nc.compile()
    return nc


def rope_tables():
    """cos/sin tables in [d, t] layout, NO sign folding (sign is in rotm)."""
    inv_freq = 1.0 / (10000.0 ** (np.arange(0, HD, 2, dtype=np.float32) / HD))
    t = np.arange(T, dtype=np.float32)
    freqs = t[:, None] * inv_freq[None, :]
    emb = np.concatenate([freqs, freqs], axis=1)  # [T, 128]
    cos = np.ascontiguousarray(np.cos(emb).T).astype(np.float32)
    sin = np.ascontiguousarray(np.sin(emb).T).astype(np.float32)
    return cos, sin


def rot_matrix():
    """rotm[k, m]: out[m] = sum_k rotm[k, m] q[k] = rotate_half(q)[m]."""
    r = np.zeros((128, 128), np.float32)
    for m in range(64):
        r[m + 64, m] = -1.0
    for m in range(64, 128):
        r[m - 64, m] = 1.0
    return r


def mask_ext():
    """[128, 256]: cols 0-127 fully masked; cols 128-255 causal triangle."""
    m = np.full((128, 256), NEGINF, np.float32)
    sl = np.arange(128)
    tl = np.arange(128)
    m[:, 128:] = np.where(sl[:, None] <= tl[None, :], 0.0, NEGINF)
    return m


def make_in_maps(x, Wq, Wk, Wv, Wo):
    scale = np.float32(1.0 / math.sqrt(HD))
    cos, sin = rope_tables()
    in_maps = []
    for c in range(8):
        b, g = c // 4, c % 4
        in_maps.append(
            {
                "xt": np.ascontiguousarray(x[b].T),
                "wq": np.ascontiguousarray(Wq[:, 512 * g : 512 * (g + 1)]) * scale,
                "wk": np.ascontiguousarray(Wk[:, 128 * g : 128 * (g + 1)]),
                "wv": np.ascontiguousarray(Wv[:, 128 * g : 128 * (g + 1)]),
                "wo": np.ascontiguousarray(Wo[512 * g : 512 * (g + 1), :]).astype(
                    ml_dtypes.bfloat16
                ),
                "cost": cos,
                "sint": sin,
                "maskx": mask_ext(),
                "identd": np.eye(128, dtype=np.float32),
                "onesd": np.ones((128, 128), np.float32),
                "rotmd": rot_matrix(),
            }
        )
    return in_maps


_CACHE = {}


def _get_nc():
    if "nc" not in _CACHE:
        _CACHE["nc"] = build_nc()
    return _CACHE["nc"]


def kernel(**inputs):
    x = np.asarray(inputs["x"], np.float32)
    Wq = np.asarray(inputs["Wq"], np.float32)
    Wk = np.asarray(inputs["Wk"], np.float32)
    Wv = np.asarray(inputs["Wv"], np.float32)
    Wo = np.asarray(inputs["Wo"], np.float32)
    in_maps = make_in_maps(x, Wq, Wk, Wv, Wo)
    nc = _get_nc()
    res = run_bass_kernel_spmd(nc, in_maps, core_ids=list(range(8)))
    outs = [np.asarray(r["y"]).astype(np.float32) for r in res.results]
    y = np.stack(
        [
            outs[0] + outs[1] + outs[2] + outs[3],
            outs[4] + outs[5] + outs[6] + outs[7],
        ]
    )
    return y.astype(np.float32)


# revision 24
# speedup vs baseline: 1.0450x; 1.0082x over previous
"""GroupedQueryAttention Trainium2 kernel (v2: transposed-S design).

Sharding: 8 cores = 2 (batch) x 4 (kv-head groups / tensor parallel).
Core c: b = c//4, g = c%4 owns q-heads 4g..4g+3 and kv-head g.
Each core computes a partial o-projection (its 512 rows of Wo); the host
sums the 4 partials per batch (the "all-reduce" of the TP group).

Device kernel per core, interleaved per 512-wide t-chunk j:
  1. proj(j): qT/kT/vT = W^T @ x^T in [head_dim, t] layout from a host
     pretransposed x^T.  v is PE-transposed to natural [s, d] layout.
  2. RoPE per chunk in [d, t] layout: the rotate-half (with sign) is a
     constant 128x128 permutation matmul on PE; cos/sin chunk tables are
     streamed from DRAM; multiplies/adds on DVE+Pool.
  3. attention per head in TRANSPOSED layout: S^T[s, t] = kT^T qT block
     matmuls; causal mask added on (extended) diagonal blocks; exp on ACT
     writes P^T straight PSUM->SBUF (softmax max-subtraction dropped:
     |S| <~ 7 here so fp32 exp is safe and the softmax ratio is exact).
     AV accumulates O^T[d, t] directly from P^T (no P transposes at all);
     the denominator accumulates via ones-vector matmuls in PSUM.
  4. 1/denom broadcast across partitions with a 1-partition ones matmul
     (no DRAM round-trip); O^T normalized to bf16.
  5. o-proj(j): y chunk = O^T^T @ Wo_shard (bf16), accumulated over the 4
     heads; partial y stored as bf16, summed on host in fp32.
"""

import math
import sys

import numpy as np
import ml_dtypes

sys.path.insert(0, "/opt/trn_rl_repo")

import concourse.bass as bass  # noqa: E402
import concourse.tile as tile  # noqa: E402
from concourse import bacc, mybir  # noqa: E402
from concourse.bass_utils import run_bass_kernel_spmd  # noqa: E402

B, T, D = 2, 2048, 2048
NH, NKV, HD = 16, 4, 128
NQ = NH // NKV  # q heads per core
KC = D // 128  # contraction chunks
NJ = T // 512  # t chunks
F32 = mybir.dt.float32
F32R = mybir.dt.float32r
BF16 = mybir.dt.bfloat16
X = mybir.AxisListType.X
EXP = mybir.ActivationFunctionType.Exp
COPY = mybir.ActivationFunctionType.Copy
NEGINF = -1.0e30


def _r(ap):
    return ap.bitcast(F32R)


def _c0_of(st, j):
    stl = st - 4 * j
    if stl < 0:
        return 0
    return (0, 128, 256, 256)[stl]


def _body(tc, xt, wq, wk, wv, wo, cost_d, sint_d, maskx_d, identd, rotmd, onesd, y_d):
    nc = tc.nc
    from contextlib import ExitStack

    with ExitStack() as ctx:
        consts = ctx.enter_context(tc.tile_pool(name="consts", bufs=1))
        wpool = ctx.enter_context(tc.tile_pool(name="wpool", bufs=1))
        kv = ctx.enter_context(tc.tile_pool(name="kv", bufs=1))
        xp = ctx.enter_context(tc.tile_pool(name="xp", bufs=8))
        qp = ctx.enter_context(tc.tile_pool(name="qp", bufs=8))
        rt = ctx.enter_context(tc.tile_pool(name="rt", bufs=3))
        cs = ctx.enter_context(tc.tile_pool(name="cs", bufs=2))
        ptp = ctx.enter_context(tc.tile_pool(name="ptp", bufs=5))
        otp = ctx.enter_context(tc.tile_pool(name="otp", bufs=6))
        ivp = ctx.enter_context(tc.tile_pool(name="ivp", bufs=1))
        ibp_pool = ctx.enter_context(tc.tile_pool(name="ibp", bufs=2))
        ysp = ctx.enter_context(tc.tile_pool(name="ysp", bufs=2))
        ps = ctx.enter_context(tc.tile_pool(name="ps", bufs=1, space="PSUM"))

        def load_x(j):
            tiles = []
            for q4 in range(4):
                xtile = xp.tile([128, 4, 512], F32R, tag="x", name=f"x{j}_{q4}")
                nc.sync.dma_start(
                    xtile,
                    xt[512 * q4 : 512 * (q4 + 1), 512 * j : 512 * (j + 1)].rearrange(
                        "(c p) m -> p c m", p=128
                    ),
                )
                tiles.append(xtile)
            return tiles

        def load_cs(j):
            cosc = cs.tile([128, 512], F32R, tag="cos", name=f"cos{j}")
            nc.sync.dma_start(cosc, cost_d[:, 512 * j : 512 * (j + 1)])
            sinc = cs.tile([128, 512], F32R, tag="sin", name=f"sin{j}")
            nc.sync.dma_start(sinc, sint_d[:, 512 * j : 512 * (j + 1)])
            return cosc, sinc

        # ---- startup loads, ordered so proj(0) can start ASAP:
        # proj m=4 (k) consumes x quarters in order, needing only wk first.
        wkt = wpool.tile([128, 16, 128], F32R, tag="wk", bufs=1, name="wkt")
        nc.sync.dma_start(
            wkt[:, 0:4, :], wk[0:512, :].rearrange("(c p) m -> p c m", p=128)
        )
        xcur = [xp.tile([128, 4, 512], F32R, tag="x", name=f"x0_{q4}") for q4 in range(4)]
        nc.sync.dma_start(
            xcur[0], xt[0:512, 0:512].rearrange("(c p) m -> p c m", p=128)
        )
        nc.sync.dma_start(
            wkt[:, 4:16, :], wk[512:2048, :].rearrange("(c p) m -> p c m", p=128)
        )
        for q4 in (1, 2, 3):
            nc.sync.dma_start(
                xcur[q4],
                xt[512 * q4 : 512 * (q4 + 1), 0:512].rearrange("(c p) m -> p c m", p=128),
            )
        wvt = wpool.tile([128, 16, 128], F32R, tag="wv", bufs=1, name="wvt")
        nc.sync.dma_start(wvt, wv.rearrange("(c p) m -> p c m", p=128))
        ident = consts.tile([128, 128], F32R, name="ident")
        nc.sync.dma_start(ident, identd)
        wqt = []
        for i in range(4):
            w = wpool.tile([128, 4, 512], F32R, tag="wq", bufs=4, name=f"wq{i}")
            nc.sync.dma_start(
                w, wq[512 * i : 512 * (i + 1), :].rearrange("(c p) m -> p c m", p=128)
            )
            wqt.append(w)
        rotm = consts.tile([128, 128], F32R, name="rotm")
        nc.sync.dma_start(rotm, rotmd)
        cscur = load_cs(0)
        maskx = consts.tile([128, 256], F32, name="maskx")
        nc.sync.dma_start(maskx, maskx_d)
        ones = consts.tile([128, 128], F32R, name="ones")
        nc.sync.dma_start(ones, onesd)
        wot = []
        for hh in range(4):
            w = wpool.tile([128, T], BF16, tag="wo", bufs=4, name=f"wo{hh}")
            nc.sync.dma_start(w, wo[128 * hh : 128 * (hh + 1), :])
            wot.append(w)

        kT = kv.tile([128, T], F32R, tag="kT", name="kT")
        vnat = kv.tile([128, T], F32R, tag="vnat", name="vnat")

        for j in range(NJ):
            jlo = 512 * j
            cosc, sinc = cscur
            qcur = [None] * 4

            # ---- proj(j) with RoPE inlined: each cross-engine dependency
            # gets a full 16-matmul block of PE slack before its consumer.
            def proj_block(m):
                pm = ps.tile([128, 512], F32, tag="big", bufs=5, name=f"pm{j}_{m}")
                for kc in range(KC):
                    if m == 4:
                        lhsT = wkt[:, kc, :]
                    elif m == 5:
                        lhsT = wvt[:, kc, :]
                    else:
                        lhsT = wqt[kc // 4][:, kc % 4, 128 * m : 128 * (m + 1)]
                    nc.tensor.matmul(
                        pm,
                        lhsT,
                        xcur[kc // 4][:, kc % 4, :],
                        start=(kc == 0),
                        stop=(kc == KC - 1),
                    )
                if m == 4:
                    nc.vector.tensor_copy(kT[:, jlo : jlo + 512], pm)
                elif m == 5:
                    vtmp_ = rt.tile([128, 512], F32R, tag="rt", name=f"vtmp{j}")
                    nc.vector.tensor_copy(vtmp_, pm)
                    return vtmp_
                else:
                    qc = qp.tile([128, 512], F32R, tag="qt", name=f"q{j}_{m}")
                    nc.scalar.activation(qc, pm, COPY)
                    qcur[m] = qc
                return None

            def rope(tgt, ri):
                rp = ps.tile([128, 512], F32, tag="big", bufs=5, name=f"rot{j}_{ri}")
                nc.tensor.matmul(rp, rotm, tgt)
                tmp = rt.tile([128, 512], F32R, tag="rt", name=f"rtmp{j}_{ri}")
                nc.vector.tensor_mul(tmp, rp, sinc)
                nc.gpsimd.tensor_mul(tgt, tgt, cosc)
                nc.vector.tensor_add(tgt, tgt, tmp)

            proj_block(4)
            vtmp = proj_block(5)
            proj_block(0)
            rope(kT[:, jlo : jlo + 512], "k")
            proj_block(1)
            rope(qcur[0], "q0")
            for c in range(4):
                tp = ps.tile([128, 128], F32, tag="big", bufs=5, name=f"vt{j}{c}")
                nc.tensor.transpose(_r(tp), vtmp[:, 128 * c : 128 * (c + 1)], ident)
                st = 4 * j + c
                nc.vector.tensor_copy(vnat[:, 128 * st : 128 * (st + 1)], tp)
            proj_block(2)
            rope(qcur[1], "q1")
            proj_block(3)

            # prefetch next chunk's x and rope tables
            if j + 1 < NJ:
                xnext = load_x(j + 1)
                csnext = load_cs(j + 1)

            # ---- attention per head, transposed layout ----
            nb = 4 * j + 4
            otcur = [None] * 4

            def emit_epilogue(pend):
                # deferred normalization: 1/den broadcast + O^T scale to bf16
                hh_, inv_p, avp_ = pend
                ibt = ps.tile([128, 512], F32, tag="big", bufs=5, name=f"ib{j}_{hh_}")
                nc.tensor.matmul(ibt, ones[0:1, :].bitcast(F32), inv_p[0:1, :])
                ivb = ibp_pool.tile([128, 512], F32, tag="invb", name=f"ivb{j}_{hh_}")
                nc.vector.tensor_copy(ivb, ibt)
                otc = otp.tile([128, 512], BF16, tag="ot", name=f"ot{j}_{hh_}")
                nc.vector.tensor_mul(otc, avp_, ivb)
                otcur[hh_] = otc

            pending = None
            for h in range(NQ):
                if h == 1:
                    rope(qcur[2], "q2")
                    rope(qcur[3], "q3")
                avp = ps.tile([128, 512], F32, tag="av", bufs=2, name=f"av{j}_{h}")
                dnp = ps.tile([128, 512], F32, tag="dn", bufs=1, name=f"dn{j}_{h}")
                pts = {}

                def emit_s(st):
                    c0 = _c0_of(st, j)
                    sp = ps.tile([128, 512], F32, tag="big", bufs=5, name=f"s{j}{h}{st}")
                    nc.tensor.matmul(
                        sp[:, c0:512],
                        kT[:, 128 * st : 128 * (st + 1)],
                        qcur[h][:, c0:512],
                    )
                    stl = st - 4 * j
                    if stl == 3:
                        nc.vector.tensor_add(sp[:, 256:512], sp[:, 256:512], maskx)
                    elif stl >= 0:
                        od = 128 * stl
                        nc.vector.tensor_add(
                            sp[:, od : od + 128], sp[:, od : od + 128], maskx[:, 128:256]
                        )
                    pt_ = ptp.tile([128, 512], F32R, tag="pt", name=f"p{j}{h}{st}")
                    nc.scalar.activation(pt_[:, c0:512], sp[:, c0:512], EXP)
                    pts[st] = pt_

                emit_s(0)
                emit_s(1)
                emit_s(2)
                emit_s(3)
                for st in range(nb):
                    if st + 4 < nb:
                        emit_s(st + 4)
                    if st == 1 and pending is not None:
                        emit_epilogue(pending)
                        pending = None
                    c0 = _c0_of(st, j)
                    nc.tensor.matmul(
                        dnp[0:1, c0:512],
                        ones[:, 0:1],
                        pts[st][:, c0:512],
                        start=(st == 0),
                        stop=(st == nb - 1),
                    )
                    nc.tensor.matmul(
                        avp[:, c0:512],
                        vnat[:, 128 * st : 128 * (st + 1)],
                        pts[st][:, c0:512],
                        start=(st == 0),
                        stop=(st == nb - 1),
                    )
                    del pts[st]

                inv_ = ivp.tile([1, 512], F32, tag="inv", name=f"inv{j}_{h}")
                nc.vector.reciprocal(inv_[0:1, :], dnp[0:1, :])
                pending = (h, inv_, avp)

            # ---- o-proj(j): y rows [jlo, jlo+512) ----
            # il=0 runs hh-major so the 4 concurrent psum accumulations absorb
            # the last head's deferred-normalization latency; later ils run
            # nch-major so each psum frees (and its copy starts) early.
            for il in range(4):
                ysb = ysp.tile([128, T], BF16, tag="ysb", name=f"y{j}_{il}")

                def ycopy(nch, yp):
                    eng = nc.vector if nch % 2 == 0 else nc.scalar
                    if eng is nc.vector:
                        nc.vector.tensor_copy(
                            ysb[:, 512 * nch : 512 * (nch + 1)], yp
                        )
                    else:
                        nc.scalar.activation(
                            ysb[:, 512 * nch : 512 * (nch + 1)], yp, COPY
                        )

                if il == 0:
                    yps = [
                        ps.tile([128, 512], F32, tag="big", bufs=5, name=f"yp{j}0{n}")
                        for n in range(4)
                    ]
                    for hh in range(3):
                        for nch in range(4):
                            nc.tensor.matmul(
                                yps[nch],
                                otcur[hh][:, 0:128],
                                wot[hh][:, 512 * nch : 512 * (nch + 1)],
                                start=(hh == 0),
                                stop=False,
                            )
                    emit_epilogue(pending)
                    pending = None
                    for nch in range(4):
                        nc.tensor.matmul(
                            yps[nch],
                            otcur[3][:, 0:128],
                            wot[3][:, 512 * nch : 512 * (nch + 1)],
                            start=False,
                            stop=True,
                        )
                    for nch in range(4):
                        ycopy(nch, yps[nch])
                else:
                    for nch in range(4):
                        yp = ps.tile(
                            [128, 512], F32, tag="big", bufs=5, name=f"yp{j}{il}{nch}"
                        )
                        for hh in range(4):
                            nc.tensor.matmul(
                                yp,
                                otcur[hh][:, 128 * il : 128 * (il + 1)],
                                wot[hh][:, 512 * nch : 512 * (nch + 1)],
                                start=(hh == 0),
                                stop=(hh == 3),
                            )
                        ycopy(nch, yp)
                if j == NJ - 1 and il == 3:
                    nc.sync.dma_start(
                        y_d[jlo + 128 * il : jlo + 128 * (il + 1), 0:1024],
                        ysb[:, 0:1024],
                    )
                    nc.sync.dma_start(
                        y_d[jlo + 128 * il : jlo + 128 * (il + 1), 1024:2048],
                        ysb[:, 1024:2048],
                    )
                else:
                    nc.sync.dma_start(
                        y_d[jlo + 128 * il : jlo + 128 * (il + 1), :], ysb
                    )

            if j + 1 < NJ:
                xcur = xnext
                cscur = csnext


def build_nc():
    nc = bacc.Bacc("TRN2", target_bir_lowering=False, debug=False, num_devices=8)
    xt = nc.dram_tensor("xt", [D, T], F32R, kind="ExternalInput").ap()
    wq = nc.dram_tensor("wq", [D, NQ * HD], F32R, kind="ExternalInput").ap()
    wk = nc.dram_tensor("wk", [D, HD], F32R, kind="ExternalInput").ap()
    wv = nc.dram_tensor("wv", [D, HD], F32R, kind="ExternalInput").ap()
    wo = nc.dram_tensor("wo", [NQ * HD, D], BF16, kind="ExternalInput").ap()
    identd = nc.dram_tensor("identd", [128, 128], F32R, kind="ExternalInput").ap()
    rotmd = nc.dram_tensor("rotmd", [128, 128], F32R, kind="ExternalInput").ap()
    onesd = nc.dram_tensor("onesd", [128, 128], F32R, kind="ExternalInput").ap()
    cost = nc.dram_tensor("cost", [HD, T], F32R, kind="ExternalInput").ap()
    sint = nc.dram_tensor("sint", [HD, T], F32R, kind="ExternalInput").ap()
    maskx = nc.dram_tensor("maskx", [128, 256], F32, kind="ExternalInput").ap()
    y = nc.dram_tensor("y", [T, D], BF16, kind="ExternalOutput").ap()
    with tile.TileContext(nc) as tc:
        _body(tc, xt, wq, wk, wv, wo, cost, sint, maskx, identd, rotmd, onesd, y)
    nc.compile()
    return nc


def rope_tables():
    """cos/sin tables in [d, t] layout, NO sign folding (sign is in rotm)."""
    inv_freq = 1.0 / (10000.0 ** (np.arange(0, HD, 2, dtype=np.float32) / HD))
    t = np.arange(T, dtype=np.float32)
    freqs = t[:, None] * inv_freq[None, :]
    emb = np.concatenate([freqs, freqs], axis=1)  # [T, 128]
    cos = np.ascontiguousarray(np.cos(emb).T).astype(np.float32)
    sin = np.ascontiguousarray(np.sin(emb).T).astype(np.float32)
    return cos, sin


def rot_matrix():
    """rotm[k, m]: out[m] = sum_k rotm[k, m] q[k] = rotate_half(q)[m]."""
    r = np.zeros((128, 128), np.float32)
    for m in range(64):
        r[m + 64, m] = -1.0
    for m in range(64, 128):
        r[m - 64, m] = 1.0
    return r


def mask_ext():
    """[128, 256]: cols 0-127 fully masked; cols 128-255 causal triangle."""
    m = np.full((128, 256), NEGINF, np.float32)
    sl = np.arange(128)
    tl = np.arange(128)
    m[:, 128:] = np.where(sl[:, None] <= tl[None, :], 0.0, NEGINF)
    return m


def make_in_maps(x, Wq, Wk, Wv, Wo):
    scale = np.float32(1.0 / math.sqrt(HD))
    cos, sin = rope_tables()
    in_maps = []
    for c in range(8):
        b, g = c // 4, c % 4
        in_maps.append(
            {
                "xt": np.ascontiguousarray(x[b].T),
                "wq": np.ascontiguousarray(Wq[:, 512 * g : 512 * (g + 1)]) * scale,
                "wk": np.ascontiguousarray(Wk[:, 128 * g : 128 * (g + 1)]),
                "wv": np.ascontiguousarray(Wv[:, 128 * g : 128 * (g + 1)]),
                "wo": np.ascontiguousarray(Wo[512 * g : 512 * (g + 1), :]).astype(
                    ml_dtypes.bfloat16
                ),
                "cost": cos,
                "sint": sin,
                "maskx": mask_ext(),
                "identd": np.eye(128, dtype=np.float32),
                "onesd": np.ones((128, 128), np.float32),
                "rotmd": rot_matrix(),
            }
        )
    return in_maps


_CACHE = {}


def _get_nc():
    if "nc" not in _CACHE:
        _CACHE["nc"] = build_nc()
    return _CACHE["nc"]


def kernel(**inputs):
    x = np.asarray(inputs["x"], np.float32)
    Wq = np.asarray(inputs["Wq"], np.float32)
    Wk = np.asarray(inputs["Wk"], np.float32)
    Wv = np.asarray(inputs["Wv"], np.float32)
    Wo = np.asarray(inputs["Wo"], np.float32)
    in_maps = make_in_maps(x, Wq, Wk, Wv, Wo)
    nc = _get_nc()
    res = run_bass_kernel_spmd(nc, in_maps, core_ids=list(range(8)))
    outs = [np.asarray(r["y"]).astype(np.float32) for r in res.results]
    y = np.stack(
        [
            outs[0] + outs[1] + outs[2] + outs[3],
            outs[4] + outs[5] + outs[6] + outs[7],
        ]
    )
    return y.astype(np.float32)
